# revision 31
# baseline (speedup 1.0000x reference)
"""Trainium2 Bass kernel for DeepRadAEVComputer (B=8, N=256).

Sharding: data-parallel over batch dim B — each of the 8 NeuronCores
processes one molecule (d [256,256], z [256]); weights replicated.

Per-core algorithm (feature-major bf16 MLP):
  mask/fc/chem features computed point-major at full 128-partition width,
  then shuffled into feature-major rhs blocks ([3,512] rows per 512-point
  group packed 4 groups per [128,512] SBUF tile at partition bases
  0/32/64/96). The 7-layer tanh MLP runs feature-major in bf16 (PE at
  1 cycle/row vs 4 for fp32) with two K=64 matmuls packed on the PE via
  disjoint (row,col) tile positions, so the per-layer psum is
  partition-stacked [128, 2048] and every tanh runs at full 128-partition
  ACT width. Supers advance in two-super lockstep so the ACT engine (the
  tanh element-throughput floor) always has the sibling super's psum to
  drain while the PE refills the slot ACT just freed. The cutoff-weighted
  sum over neighbors j is a DMA partition-broadcast of w followed by a
  segmented DVE reduce; the final row-normalize runs per output half (the
  first half hides under the pipeline) with a PE ones-matvec for the
  partition sum and a PE transpose for a contiguous store. Weights arrive
  pre-stacked/pre-cast to bf16 from the host (layout prep only), so no
  on-device staging conversions gate the pipeline start. ACT Sin is
  inaccurate on TRN2, so the cutoff cosine is evaluated as an even
  minimax polynomial on the DVE.
"""

import math
import sys

import numpy as np

sys.path.insert(0, "/opt/trn_rl_repo")

import ml_dtypes  # noqa: E402

import concourse.bass as bass  # noqa: E402
import concourse.tile as tile  # noqa: E402
from concourse import bacc, masks, mybir  # noqa: E402
from concourse.bass_utils import run_bass_kernel_spmd  # noqa: E402

F32 = mybir.dt.float32
BF16 = mybir.dt.bfloat16
I32 = mybir.dt.int32
AF = mybir.ActivationFunctionType
OP = mybir.AluOpType

N = 256          # atoms per molecule
NPTS = N * N     # 65536 pair-points per core
GSZ = 512        # points per group (2 d-rows)
NGRP = NPTS // GSZ          # 128 groups
SUPER = 8                   # groups per super-tile
NSUP = NGRP // SUPER        # 16 super-tiles
CUTOFF = 5.2
EPS = 1e-7
PI = math.pi

DIMS = [(3, 64), (64, 64), (64, 64), (64, 64), (64, 64), (64, 64), (64, 128)]

# cos(u)-1 = sum_{k>=1} g_k (u^2)^k minimax on [-pi, pi] (max err 4.6e-10)
GCOS = [
    -0.49999999920407, 0.041666664531272414, -0.0013888869362829136,
    2.4800740359237187e-05, -2.7537667614463017e-07, 2.0625484749459024e-09,
    -9.788307045326216e-12,
]

_BUILT = None


def _build():
    nc = bacc.Bacc("TRN2", target_bir_lowering=False, debug=False)

    d_in = nc.dram_tensor("d", [N, N], F32, kind="ExternalInput")
    dbf_in = nc.dram_tensor("dbf", [N, N], BF16, kind="ExternalInput")
    zbc_in = nc.dram_tensor("zbc", [128, N], F32, kind="ExternalInput")
    zf_in = nc.dram_tensor("zf", [N], F32, kind="ExternalInput")
    w_dram = []
    b_dram = []
    for li, (fi, fo) in enumerate(DIMS):
        w_dram.append(nc.dram_tensor(f"Wb{li}", [128, fo], BF16, kind="ExternalInput"))
        b_dram.append(nc.dram_tensor(f"bb{li}", [128, 1], F32, kind="ExternalInput"))
    out_dram = nc.dram_tensor("out", [N, 128], F32, kind="ExternalOutput")

    dbf_flat = dbf_in[:].rearrange("a b -> (a b)")

    with tile.TileContext(nc) as tc:
        with (
            tc.tile_pool(name="const", bufs=1) as cpool,
            tc.tile_pool(name="prep", bufs=1) as ppool,
            tc.tile_pool(name="scratch", bufs=2) as spool,
            tc.tile_pool(name="rhs", bufs=6) as rpool,
            tc.tile_pool(name="wb", bufs=6) as wpool,
            tc.tile_pool(name="hbuf", bufs=20) as hpool,
            tc.tile_pool(name="psum", bufs=2, space="PSUM") as qpool,
        ):
            # ---- constants ----
            ones_t = cpool.tile([128, 128], F32)
            nc.gpsimd.memset(ones_t[:], 1.0)
            ident = cpool.tile([128, 128], F32)
            masks.make_identity(nc, ident[:])
            eps12 = cpool.tile([128, 1], F32)
            nc.gpsimd.memset(eps12[:], 1e-12)

            # prep inputs first on the sync queue so the DVE feature chain
            # starts immediately; weights follow (first needed only by L0).
            zbc = ppool.tile([128, N], F32, tag="zbc")
            nc.sync.dma_start(zbc[:], zbc_in[:])

            c1h = []
            c2h = []
            wh = []

            def prep_half(hf):
                zcol = ppool.tile([128, 1], F32, tag=f"zcol{hf}")
                nc.sync.dma_start(
                    zcol[:],
                    zf_in[128 * hf : 128 * hf + 128].rearrange("(p f) -> p f", f=1),
                )
                dh = spool.tile([128, N], F32)
                nc.sync.dma_start(dh[:], d_in[128 * hf : 128 * hf + 128, :])

                s = spool.tile([128, N], F32)
                nc.vector.tensor_scalar_add(s[:], zbc[:], zcol[:])
                m = spool.tile([128, N], F32)
                nc.vector.tensor_scalar_mul(m[:], zbc[:], zcol[:])
                q = spool.tile([128, N], F32)
                nc.vector.tensor_tensor(q[:], s[:], s[:], OP.mult)
                m2 = spool.tile([128, N], F32)
                nc.vector.tensor_tensor(m2[:], m[:], m[:], OP.mult)
                nc.vector.tensor_tensor(q[:], q[:], m2[:], OP.add)
                sq = spool.tile([128, N], F32)
                nc.scalar.activation(sq[:], q[:], AF.Sqrt, bias=eps12[:])
                fac = spool.tile([128, N], F32)
                nc.vector.reciprocal(fac[:], sq[:])
                c1 = ppool.tile([128, N], BF16, tag=f"c1_{hf}")
                nc.vector.tensor_tensor(c1[:], s[:], fac[:], OP.mult)
                c2 = ppool.tile([128, N], BF16, tag=f"c2_{hf}")
                nc.vector.tensor_tensor(c2[:], m[:], fac[:], OP.mult)

                # w = mask * (0.5*cos(pi*d/CUTOFF)+0.5) via DVE polynomial
                th = spool.tile([128, N], F32)
                nc.vector.tensor_single_scalar(th[:], dh[:], PI / CUTOFF, OP.mult)
                uu = spool.tile([128, N], F32)
                nc.vector.tensor_single_scalar(uu[:], th[:], PI, OP.subtract)
                vv = spool.tile([128, N], F32)
                nc.vector.tensor_tensor(vv[:], uu[:], uu[:], OP.mult)
                pa = spool.tile([128, N], F32)
                pb = spool.tile([128, N], F32)
                nc.vector.tensor_single_scalar(pa[:], vv[:], GCOS[6], OP.mult)
                cur, nxt = pa, pb
                for k in range(5, -1, -1):
                    nc.vector.scalar_tensor_tensor(
                        nxt[:], cur[:], GCOS[k], vv[:], OP.add, OP.mult
                    )
                    cur, nxt = nxt, cur
                ne = spool.tile([128, N], F32)
                nc.vector.tensor_single_scalar(ne[:], dh[:], 0.0, OP.not_equal)
                msk = spool.tile([128, N], F32)
                nc.vector.scalar_tensor_tensor(
                    msk[:], dh[:], CUTOFF, ne[:], OP.is_lt, OP.mult
                )
                fcb = spool.tile([128, N], F32)
                nc.vector.tensor_single_scalar(fcb[:], cur[:], -0.5, OP.mult)
                wv = ppool.tile([128, N], BF16, tag=f"w_{hf}")
                nc.vector.tensor_tensor(wv[:], fcb[:], msk[:], OP.mult)
                c1h.append(c1)
                c2h.append(c2)
                wh.append(wv)

            # half 0 feeds supers 0-7; half 1 is emitted inside pair 1 so its
            # DVE chain runs under the pipeline without gating the start.
            prep_half(0)

            # weights/biases arrive pre-stacked (both 64-partition halves /
            # four 32-partition strips for W0) and pre-cast to bf16
            wt = []
            bt = []
            for li, (fi, fo) in enumerate(DIMS):
                t = cpool.tile([128, fo], BF16, tag=f"W{li}")
                nc.sync.dma_start(t[:], w_dram[li][:])
                wt.append(t)
                b = cpool.tile([128, 1], F32, tag=f"b{li}")
                nc.sync.dma_start(b[:], b_dram[li][:])
                bt.append(b)

            # GR^T accumulator [feat, i]
            grt = ppool.tile([128, N], F32, tag="grt")

            def build_blocks(sidx):
                # L0 rhs blocks: 4 groups per [128,512] tile,
                # feature rows at partitions 32k+{0,1,2}
                blks = []
                wblks = []
                for bi in range(2):
                    blk = rpool.tile([128, GSZ], BF16, tag="rhs0")
                    c = 2 * sidx + bi  # block index over 32
                    view = blk[:].rearrange("(k r) f -> k r f", r=32)
                    nc.gpsimd.dma_start(
                        view[:, 0:1, :],
                        dbf_flat[2048 * c : 2048 * (c + 1)].rearrange(
                            "(g f) -> g f", f=GSZ
                        ),
                    )
                    # block c covers groups 4c..4c+3 => i-rows 8c..8c+7
                    hf = (8 * c) // 128
                    r0 = (8 * c) % 128
                    for feat, src in ((1, c1h[hf]), (2, c2h[hf])):
                        nc.gpsimd.dma_start(
                            view[:, feat : feat + 1, :], src[r0 : r0 + 8, :]
                        )
                    blks.append(blk)

                    # w broadcast tile: row 0 = this half-super's 8 i-rows of
                    # w, then log2 partition-doubling copies on the gpsimd
                    # DMA queue (the sync queue carries the d/weight loads)
                    wb = wpool.tile([128, 2048], BF16, tag="wb")
                    nc.gpsimd.dma_start(wb[0:1, :], wh[hf][r0 : r0 + 8, :])
                    for k in (1, 2, 4, 8, 16, 32, 64):
                        nc.gpsimd.dma_start(wb[k : 2 * k, :], wb[0:k, :])
                    wblks.append(wb)
                return blks, wblks

            def mm_layer(ps, rhs_t, w_tile):
                # 8 matmuls: group j in super; pair p=j//2, half q=j%2
                for j in range(SUPER):
                    p, qh = j // 2, j % 2
                    nc.tensor.matmul(
                        ps[64 * qh : 64 * qh + 64, GSZ * p : GSZ * (p + 1)],
                        w_tile[64 * qh : 64 * qh + 64, :],
                        rhs_t[
                            64 * qh : 64 * qh + 64,
                            GSZ * p : GSZ * (p + 1),
                        ],
                    )

            def act_tanh(dst, ps, bias):
                nc.scalar.activation(dst[:], ps[:], AF.Tanh, bias=bias[:])

            def do_l0(blks):
                ps0 = qpool.tile([128, 2048], F32, tag="ps")
                for j in range(SUPER):
                    bi, k = j // 4, j % 4
                    p, qh = j // 2, j % 2
                    nc.tensor.matmul(
                        ps0[64 * qh : 64 * qh + 64, GSZ * p : GSZ * (p + 1)],
                        wt[0][32 * k : 32 * k + 3, :],
                        blks[bi][32 * k : 32 * k + 3, :],
                        tile_position=(32 * k, 64 * qh),
                    )
                h0 = hpool.tile([128, 2048], BF16, tag="hbuf")
                act_tanh(h0, ps0, bt[0])
                return h0

            def hidden(rhs_t, li):
                ps = qpool.tile([128, 2048], F32, tag="ps")
                mm_layer(ps, rhs_t, wt[li])
                h = hpool.tile([128, 2048], BF16, tag="hbuf")
                act_tanh(h, ps, bt[li])
                return h

            def resid(a, b):
                r = hpool.tile([128, 2048], BF16, tag="hbuf")
                nc.vector.tensor_tensor(r[:], a[:], b[:], OP.add)
                return r

            def l6_half(r3_t, wblk, half, sidx):
                ps6 = qpool.tile([128, 2048], F32, tag="ps")
                for jj in range(4):
                    j = 4 * half + jj
                    qh = j % 2
                    nc.tensor.matmul(
                        ps6[:, GSZ * jj : GSZ * (jj + 1)],
                        wt[6][64 * qh : 64 * qh + 64, :],
                        r3_t[
                            64 * qh : 64 * qh + 64,
                            GSZ * (j // 2) : GSZ * (j // 2 + 1),
                        ],
                    )
                h6 = hpool.tile([128, 2048], BF16, tag="hbuf")
                act_tanh(h6, ps6, bt[6])
                prod = hpool.tile([128, 2048], BF16, tag="hbuf")
                nc.vector.tensor_tensor(prod[:], h6[:], wblk[:], OP.mult)
                col = 16 * sidx + 8 * half
                nc.vector.tensor_reduce(
                    grt[:, col : col + 8],
                    prod[:].rearrange("p (s x) -> p s x", x=N),
                    mybir.AxisListType.X,
                    OP.add,
                )

            def tail_half(t):
                # normalize + transpose + store output rows 128t..128t+127;
                # half 0 is complete after pair 3 and runs under the pipeline
                g = grt[:, 128 * t : 128 * t + 128]
                gsq = spool.tile([128, 128], F32, tag=f"tq{t}")
                nc.vector.tensor_tensor(gsq[:], g, g, OP.mult)
                ps_n = qpool.tile([128, 2048], F32, tag="ps")
                nc.tensor.matmul(ps_n[0:1, 0:128], ones_t[:, 0:1], gsq[:])
                nrm = spool.tile([1, 128], F32, tag=f"tn{t}")
                nc.scalar.activation(nrm[:], ps_n[0:1, 0:128], AF.Sqrt)
                nc.vector.tensor_single_scalar(nrm[:], nrm[:], EPS, OP.add)
                inv = spool.tile([1, 128], F32, tag=f"ti{t}")
                nc.vector.reciprocal(inv[:], nrm[:])
                ps_b = qpool.tile([128, 2048], F32, tag="ps")
                nc.tensor.matmul(ps_b[:, 0:128], ones_t[0:1, :], inv[:])
                grn = spool.tile([128, 128], F32, tag=f"tg{t}")
                nc.vector.tensor_tensor(grn[:], g, ps_b[:, 0:128], OP.mult)
                ps_t = qpool.tile([128, 2048], F32, tag="ps")
                nc.tensor.transpose(ps_t[:, 0:128], grn[:], ident[:])
                ot = spool.tile([128, 128], F32, tag=f"to{t}")
                nc.scalar.copy(ot[:], ps_t[:, 0:128])
                nc.sync.dma_start(out_dram[128 * t : 128 * t + 128, :], ot[:])

            # Two-super lockstep pipeline: supers (2p, 2p+1) advance layer by
            # layer in alternation, so ACT always has the sibling super's psum
            # to drain while the PE refills the slot ACT just freed. Keeps the
            # ACT engine (the per-element tanh floor) near 100% busy and the
            # PE dense enough that the HAM clock stays warm.
            blkA, wbA = build_blocks(0)
            blkB, wbB = build_blocks(1)
            h0A = do_l0(blkA)
            h0B = do_l0(blkB)
            h1A = hidden(h0A, 1)
            h1B = hidden(h0B, 1)
            r1A = resid(h1A, h0A)
            r1B = resid(h1B, h0B)
            NPAIR = NSUP // 2
            for pair in range(NPAIR):
                sA, sB = 2 * pair, 2 * pair + 1
                h2A = hidden(r1A, 2)
                h2B = hidden(r1B, 2)
                if pair + 1 < NPAIR:
                    nblkA, nwbA = build_blocks(2 * pair + 2)
                    nblkB, nwbB = build_blocks(2 * pair + 3)
                if pair == 1:
                    # half-1 features, hidden under pairs 1-2 (first consumed
                    # by build_blocks(8) at pair 3)
                    prep_half(1)
                h3A = hidden(h2A, 3)
                h3B = hidden(h2B, 3)
                r2A = resid(h3A, r1A)
                r2B = resid(h3B, r1B)
                h4A = hidden(r2A, 4)
                h4B = hidden(r2B, 4)
                h5A = hidden(h4A, 5)
                h5B = hidden(h4B, 5)
                r3A = resid(h5A, r2A)
                r3B = resid(h5B, r2B)
                l6_half(r3A, wbA[0], 0, sA)
                l6_half(r3B, wbB[0], 0, sB)
                if pair + 1 < NPAIR:
                    h0A = do_l0(nblkA)
                l6_half(r3A, wbA[1], 1, sA)
                if pair + 1 < NPAIR:
                    h0B = do_l0(nblkB)
                l6_half(r3B, wbB[1], 1, sB)
                if pair == 3:
                    tail_half(0)
                if pair + 1 < NPAIR:
                    h1A = hidden(h0A, 1)
                    h1B = hidden(h0B, 1)
                    r1A = resid(h1A, h0A)
                    r1B = resid(h1B, h0B)
                    blkA, wbA, blkB, wbB = nblkA, nwbA, nblkB, nwbB
            tail_half(1)

    nc.compile()
    return nc


def _get_built():
    global _BUILT
    if _BUILT is None:
        _BUILT = _build()
    return _BUILT


def _host_prep(inputs):
    """Pre-stack/pre-cast the replicated weights and z layout (host-side
    input layout prep; all d-dependent compute stays on device)."""
    bf16 = ml_dtypes.bfloat16
    wmaps = {}
    for li, (fi, fo) in enumerate(DIMS):
        w = np.asarray(inputs[f"W{li}"], np.float32)
        ws = np.zeros((128, fo), dtype=bf16)
        if li == 0:
            for k in range(4):
                ws[32 * k : 32 * k + 3, :] = w.astype(bf16)
        else:
            ws[0:64, :] = w.astype(bf16)
            ws[64:128, :] = w.astype(bf16)
        b = np.asarray(inputs[f"b{li}"], np.float32)
        bs = np.zeros((128, 1), dtype=np.float32)
        if fo == 64:
            bs[0:64, 0] = b
            bs[64:128, 0] = b
        else:
            bs[:, 0] = b
        wmaps[f"Wb{li}"] = ws
        wmaps[f"bb{li}"] = bs
    return wmaps


def make_in_maps(inputs):
    d = np.ascontiguousarray(np.asarray(inputs["distance_matrices_batch"], np.float32))
    z = np.ascontiguousarray(np.asarray(inputs["atomic_numbers_batch"], np.int32))
    B = d.shape[0]
    wmaps = _host_prep(inputs)
    bf16 = ml_dtypes.bfloat16
    in_maps = []
    for c in range(B):
        zf = z[c].astype(np.float32)
        m = {
            "d": d[c],
            "dbf": d[c].astype(bf16),
            "zbc": np.ascontiguousarray(np.broadcast_to(zf[None, :], (128, N))),
            "zf": zf,
        }
        m.update(wmaps)
        in_maps.append(m)
    return in_maps


def kernel(**inputs):
    nc = _get_built()
    in_maps = make_in_maps(inputs)
    B = len(in_maps)
    res = run_bass_kernel_spmd(nc, in_maps, list(range(B)))
    return np.stack([res.results[c]["out"] for c in range(B)], 0)


# revision 32
# speedup vs baseline: 1.1674x; 1.1674x over previous
"""Trainium2 Bass kernel for DeepRadAEVComputer (B=8, N=256).

Sharding: data-parallel over batch dim B — each of the 8 NeuronCores
processes one molecule (d [256,256], z [256]); weights replicated.

Per-core algorithm (feature-major bf16 MLP):
  mask/fc/chem features computed point-major at full 128-partition width,
  then shuffled into feature-major rhs blocks ([3,512] rows per 512-point
  group packed 4 groups per [128,512] SBUF tile at partition bases
  0/32/64/96). The 7-layer tanh MLP runs feature-major in bf16 (PE at
  1 cycle/row vs 4 for fp32) with two K=64 matmuls packed on the PE via
  disjoint (row,col) tile positions, so the per-layer psum is
  partition-stacked [128, 2048] and every tanh runs at full 128-partition
  ACT width. Supers advance in two-super lockstep so the ACT engine (the
  tanh element-throughput floor) always has the sibling super's psum to
  drain while the PE refills the slot ACT just freed. The cutoff-weighted
  sum over neighbors j is a DMA partition-broadcast of w followed by a
  segmented DVE reduce; the final row-normalize runs per output half (the
  first half hides under the pipeline) with a PE ones-matvec for the
  partition sum and a PE transpose for a contiguous store. Weights arrive
  pre-stacked/pre-cast to bf16 from the host (layout prep only), so no
  on-device staging conversions gate the pipeline start. ACT Sin is
  inaccurate on TRN2, so the cutoff cosine is evaluated as an even
  minimax polynomial on the DVE.
"""

import math
import sys

import numpy as np

sys.path.insert(0, "/opt/trn_rl_repo")

import ml_dtypes  # noqa: E402

import concourse.bass as bass  # noqa: E402
import concourse.tile as tile  # noqa: E402
from concourse import bacc, masks, mybir  # noqa: E402
from concourse.bass_utils import run_bass_kernel_spmd  # noqa: E402

F32 = mybir.dt.float32
BF16 = mybir.dt.bfloat16
I32 = mybir.dt.int32
AF = mybir.ActivationFunctionType
OP = mybir.AluOpType

N = 256          # atoms per molecule
NPTS = N * N     # 65536 pair-points per core
GSZ = 512        # points per group (2 d-rows)
NGRP = NPTS // GSZ          # 128 groups
SUPER = 8                   # groups per super-tile
NSUP = NGRP // SUPER        # 16 super-tiles
CUTOFF = 5.2
EPS = 1e-7
PI = math.pi

DIMS = [(3, 64), (64, 64), (64, 64), (64, 64), (64, 64), (64, 64), (64, 128)]

# cos(u)-1 = sum_{k>=1} g_k (u^2)^k minimax on [-pi, pi] (max err 4.6e-10)
GCOS = [
    -0.49999999920407, 0.041666664531272414, -0.0013888869362829136,
    2.4800740359237187e-05, -2.7537667614463017e-07, 2.0625484749459024e-09,
    -9.788307045326216e-12,
]

_BUILT = None


def _build():
    nc = bacc.Bacc("TRN2", target_bir_lowering=False, debug=False)

    d_in = nc.dram_tensor("d", [N, N], F32, kind="ExternalInput")
    dbf_in = nc.dram_tensor("dbf", [N, N], BF16, kind="ExternalInput")
    zbc_in = nc.dram_tensor("zbc", [128, N], F32, kind="ExternalInput")
    zf_in = nc.dram_tensor("zf", [N], F32, kind="ExternalInput")
    w_dram = []
    b_dram = []
    for li, (fi, fo) in enumerate(DIMS):
        w_dram.append(nc.dram_tensor(f"Wb{li}", [128, fo], BF16, kind="ExternalInput"))
        b_dram.append(nc.dram_tensor(f"bb{li}", [128, 1], F32, kind="ExternalInput"))
    out_dram = nc.dram_tensor("out", [N, 128], F32, kind="ExternalOutput")

    dbf_flat = dbf_in[:].rearrange("a b -> (a b)")

    with tile.TileContext(nc) as tc:
        with (
            tc.tile_pool(name="const", bufs=1) as cpool,
            tc.tile_pool(name="prep", bufs=1) as ppool,
            tc.tile_pool(name="scratch", bufs=2) as spool,
            tc.tile_pool(name="rhs", bufs=6) as rpool,
            tc.tile_pool(name="wb", bufs=6) as wpool,
            tc.tile_pool(name="hbuf", bufs=20) as hpool,
            tc.tile_pool(name="psum", bufs=2, space="PSUM") as qpool,
        ):
            # ---- constants ----
            ones_t = cpool.tile([128, 128], F32)
            nc.gpsimd.memset(ones_t[:], 1.0)
            ident = cpool.tile([128, 128], F32)
            masks.make_identity(nc, ident[:])
            eps12 = cpool.tile([128, 1], F32)
            nc.gpsimd.memset(eps12[:], 1e-12)

            # prep inputs first on the sync queue so the DVE feature chain
            # starts immediately; weights follow (first needed only by L0).
            zbc = ppool.tile([128, N], F32, tag="zbc")
            nc.sync.dma_start(zbc[:], zbc_in[:])

            c1h = []
            c2h = []
            wh = []

            def prep_half(hf):
                zcol = ppool.tile([128, 1], F32, tag=f"zcol{hf}")
                nc.sync.dma_start(
                    zcol[:],
                    zf_in[128 * hf : 128 * hf + 128].rearrange("(p f) -> p f", f=1),
                )
                dh = spool.tile([128, N], F32)
                nc.sync.dma_start(dh[:], d_in[128 * hf : 128 * hf + 128, :])

                s = spool.tile([128, N], F32)
                nc.vector.tensor_scalar_add(s[:], zbc[:], zcol[:])
                m = spool.tile([128, N], F32)
                nc.vector.tensor_scalar_mul(m[:], zbc[:], zcol[:])
                q = spool.tile([128, N], F32)
                nc.vector.tensor_tensor(q[:], s[:], s[:], OP.mult)
                m2 = spool.tile([128, N], F32)
                nc.vector.tensor_tensor(m2[:], m[:], m[:], OP.mult)
                nc.vector.tensor_tensor(q[:], q[:], m2[:], OP.add)
                sq = spool.tile([128, N], F32)
                nc.scalar.activation(sq[:], q[:], AF.Sqrt, bias=eps12[:])
                fac = spool.tile([128, N], F32)
                nc.vector.reciprocal(fac[:], sq[:])
                c1 = ppool.tile([128, N], BF16, tag=f"c1_{hf}")
                nc.vector.tensor_tensor(c1[:], s[:], fac[:], OP.mult)
                c2 = ppool.tile([128, N], BF16, tag=f"c2_{hf}")
                nc.vector.tensor_tensor(c2[:], m[:], fac[:], OP.mult)

                # w = mask * (0.5*cos(pi*d/CUTOFF)+0.5) via DVE polynomial
                th = spool.tile([128, N], F32)
                nc.vector.tensor_single_scalar(th[:], dh[:], PI / CUTOFF, OP.mult)
                uu = spool.tile([128, N], F32)
                nc.vector.tensor_single_scalar(uu[:], th[:], PI, OP.subtract)
                vv = spool.tile([128, N], F32)
                nc.vector.tensor_tensor(vv[:], uu[:], uu[:], OP.mult)
                pa = spool.tile([128, N], F32)
                pb = spool.tile([128, N], F32)
                nc.vector.tensor_single_scalar(pa[:], vv[:], GCOS[6], OP.mult)
                cur, nxt = pa, pb
                for k in range(5, -1, -1):
                    nc.vector.scalar_tensor_tensor(
                        nxt[:], cur[:], GCOS[k], vv[:], OP.add, OP.mult
                    )
                    cur, nxt = nxt, cur
                ne = spool.tile([128, N], F32)
                nc.vector.tensor_single_scalar(ne[:], dh[:], 0.0, OP.not_equal)
                msk = spool.tile([128, N], F32)
                nc.vector.scalar_tensor_tensor(
                    msk[:], dh[:], CUTOFF, ne[:], OP.is_lt, OP.mult
                )
                fcb = spool.tile([128, N], F32)
                nc.vector.tensor_single_scalar(fcb[:], cur[:], -0.5, OP.mult)
                wv = ppool.tile([128, N], BF16, tag=f"w_{hf}")
                nc.vector.tensor_tensor(wv[:], fcb[:], msk[:], OP.mult)
                c1h.append(c1)
                c2h.append(c2)
                wh.append(wv)

            # half 0 feeds supers 0-7; half 1 is emitted inside pair 1 so its
            # DVE chain runs under the pipeline without gating the start.
            prep_half(0)

            # weights/biases arrive pre-stacked (both 64-partition halves /
            # four 32-partition strips for W0) and pre-cast to bf16
            wt = []
            bt = []
            for li, (fi, fo) in enumerate(DIMS):
                t = cpool.tile([128, fo], BF16, tag=f"W{li}")
                nc.sync.dma_start(t[:], w_dram[li][:])
                wt.append(t)
                b = cpool.tile([128, 1], F32, tag=f"b{li}")
                nc.sync.dma_start(b[:], b_dram[li][:])
                bt.append(b)

            # GR^T accumulator [feat, i]
            grt = ppool.tile([128, N], F32, tag="grt")

            def build_blocks(sidx):
                # L0 rhs blocks: 4 groups per [128,512] tile,
                # feature rows at partitions 32k+{0,1,2}. Both blocks' strips
                # are issued before the (later-needed) wb chains so L0 fills
                # never queue behind them.
                blks = []
                wblks = []
                for bi in range(2):
                    blk = rpool.tile([128, GSZ], BF16, tag="rhs0")
                    c = 2 * sidx + bi  # block index over 32
                    view = blk[:].rearrange("(k r) f -> k r f", r=32)
                    nc.gpsimd.dma_start(
                        view[:, 0:1, :],
                        dbf_flat[2048 * c : 2048 * (c + 1)].rearrange(
                            "(g f) -> g f", f=GSZ
                        ),
                    )
                    # block c covers groups 4c..4c+3 => i-rows 8c..8c+7
                    hf = (8 * c) // 128
                    r0 = (8 * c) % 128
                    for feat, src in ((1, c1h[hf]), (2, c2h[hf])):
                        nc.gpsimd.dma_start(
                            view[:, feat : feat + 1, :], src[r0 : r0 + 8, :]
                        )
                    blks.append(blk)
                for bi in range(2):
                    c = 2 * sidx + bi
                    hf = (8 * c) // 128
                    r0 = (8 * c) % 128
                    # w broadcast tile: row 0 = this half-super's 8 i-rows of
                    # w, then log2 partition-doubling copies on the sync DMA
                    # queue (keeps the gpsimd queue free for block strips)
                    wb = wpool.tile([128, 2048], BF16, tag="wb")
                    nc.sync.dma_start(wb[0:1, :], wh[hf][r0 : r0 + 8, :])
                    for k in (1, 2, 4, 8, 16, 32, 64):
                        nc.sync.dma_start(wb[k : 2 * k, :], wb[0:k, :])
                    wblks.append(wb)
                return blks, wblks

            def mm_layer(ps, rhs_t, w_tile):
                # 8 matmuls: group j in super; pair p=j//2, half q=j%2
                for j in range(SUPER):
                    p, qh = j // 2, j % 2
                    nc.tensor.matmul(
                        ps[64 * qh : 64 * qh + 64, GSZ * p : GSZ * (p + 1)],
                        w_tile[64 * qh : 64 * qh + 64, :],
                        rhs_t[
                            64 * qh : 64 * qh + 64,
                            GSZ * p : GSZ * (p + 1),
                        ],
                    )

            def act_tanh(dst, ps, bias):
                nc.scalar.activation(dst[:], ps[:], AF.Tanh, bias=bias[:])

            def do_l0(blks):
                ps0 = qpool.tile([128, 2048], F32, tag="ps")
                for j in range(SUPER):
                    bi, k = j // 4, j % 4
                    p, qh = j // 2, j % 2
                    nc.tensor.matmul(
                        ps0[64 * qh : 64 * qh + 64, GSZ * p : GSZ * (p + 1)],
                        wt[0][32 * k : 32 * k + 3, :],
                        blks[bi][32 * k : 32 * k + 3, :],
                        tile_position=(32 * k, 64 * qh),
                    )
                h0 = hpool.tile([128, 2048], BF16, tag="hbuf")
                act_tanh(h0, ps0, bt[0])
                return h0

            def hidden(rhs_t, li):
                ps = qpool.tile([128, 2048], F32, tag="ps")
                mm_layer(ps, rhs_t, wt[li])
                h = hpool.tile([128, 2048], BF16, tag="hbuf")
                act_tanh(h, ps, bt[li])
                return h

            def resid(a, b):
                r = hpool.tile([128, 2048], BF16, tag="hbuf")
                nc.vector.tensor_tensor(r[:], a[:], b[:], OP.add)
                return r

            def l6_half(r3_t, wblk, half, sidx):
                ps6 = qpool.tile([128, 2048], F32, tag="ps")
                for jj in range(4):
                    j = 4 * half + jj
                    qh = j % 2
                    nc.tensor.matmul(
                        ps6[:, GSZ * jj : GSZ * (jj + 1)],
                        wt[6][64 * qh : 64 * qh + 64, :],
                        r3_t[
                            64 * qh : 64 * qh + 64,
                            GSZ * (j // 2) : GSZ * (j // 2 + 1),
                        ],
                    )
                h6 = hpool.tile([128, 2048], BF16, tag="hbuf")
                act_tanh(h6, ps6, bt[6])
                prod = hpool.tile([128, 2048], BF16, tag="hbuf")
                nc.vector.tensor_tensor(prod[:], h6[:], wblk[:], OP.mult)
                col = 16 * sidx + 8 * half
                nc.vector.tensor_reduce(
                    grt[:, col : col + 8],
                    prod[:].rearrange("p (s x) -> p s x", x=N),
                    mybir.AxisListType.X,
                    OP.add,
                )

            def tail_half(t):
                # normalize + transpose + store output rows 128t..128t+127;
                # half 0 is complete after pair 3 and runs under the pipeline
                g = grt[:, 128 * t : 128 * t + 128]
                gsq = spool.tile([128, 128], F32, tag=f"tq{t}")
                nc.vector.tensor_tensor(gsq[:], g, g, OP.mult)
                ps_n = qpool.tile([128, 2048], F32, tag="ps")
                nc.tensor.matmul(ps_n[0:1, 0:128], ones_t[:, 0:1], gsq[:])
                nrm = spool.tile([1, 128], F32, tag=f"tn{t}")
                nc.scalar.activation(nrm[:], ps_n[0:1, 0:128], AF.Sqrt)
                nc.vector.tensor_single_scalar(nrm[:], nrm[:], EPS, OP.add)
                inv = spool.tile([1, 128], F32, tag=f"ti{t}")
                nc.vector.reciprocal(inv[:], nrm[:])
                ps_b = qpool.tile([128, 2048], F32, tag="ps")
                nc.tensor.matmul(ps_b[:, 0:128], ones_t[0:1, :], inv[:])
                grn = spool.tile([128, 128], F32, tag=f"tg{t}")
                nc.vector.tensor_tensor(grn[:], g, ps_b[:, 0:128], OP.mult)
                ps_t = qpool.tile([128, 2048], F32, tag="ps")
                nc.tensor.transpose(ps_t[:, 0:128], grn[:], ident[:])
                ot = spool.tile([128, 128], F32, tag=f"to{t}")
                nc.scalar.copy(ot[:], ps_t[:, 0:128])
                nc.sync.dma_start(out_dram[128 * t : 128 * t + 128, :], ot[:])

            # Two-super lockstep pipeline: supers (2p, 2p+1) advance layer by
            # layer in alternation, so ACT always has the sibling super's psum
            # to drain while the PE refills the slot ACT just freed. Keeps the
            # ACT engine (the per-element tanh floor) near 100% busy and the
            # PE dense enough that the HAM clock stays warm.
            blkA, wbA = build_blocks(0)
            blkB, wbB = build_blocks(1)
            h0A = do_l0(blkA)
            h0B = do_l0(blkB)
            h1A = hidden(h0A, 1)
            h1B = hidden(h0B, 1)
            r1A = resid(h1A, h0A)
            r1B = resid(h1B, h0B)
            NPAIR = NSUP // 2
            for pair in range(NPAIR):
                sA, sB = 2 * pair, 2 * pair + 1
                h2A = hidden(r1A, 2)
                h2B = hidden(r1B, 2)
                if pair + 1 < NPAIR:
                    nblkA, nwbA = build_blocks(2 * pair + 2)
                    nblkB, nwbB = build_blocks(2 * pair + 3)
                if pair == 1:
                    # half-1 features, hidden under pairs 1-2 (first consumed
                    # by build_blocks(8) at pair 3)
                    prep_half(1)
                h3A = hidden(h2A, 3)
                h3B = hidden(h2B, 3)
                r2A = resid(h3A, r1A)
                r2B = resid(h3B, r1B)
                h4A = hidden(r2A, 4)
                h4B = hidden(r2B, 4)
                h5A = hidden(h4A, 5)
                h5B = hidden(h4B, 5)
                r3A = resid(h5A, r2A)
                r3B = resid(h5B, r2B)
                l6_half(r3A, wbA[0], 0, sA)
                l6_half(r3B, wbB[0], 0, sB)
                if pair + 1 < NPAIR:
                    h0A = do_l0(nblkA)
                l6_half(r3A, wbA[1], 1, sA)
                if pair + 1 < NPAIR:
                    h0B = do_l0(nblkB)
                l6_half(r3B, wbB[1], 1, sB)
                if pair == 3:
                    tail_half(0)
                if pair + 1 < NPAIR:
                    h1A = hidden(h0A, 1)
                    h1B = hidden(h0B, 1)
                    r1A = resid(h1A, h0A)
                    r1B = resid(h1B, h0B)
                    blkA, wbA, blkB, wbB = nblkA, nwbA, nblkB, nwbB
            tail_half(1)

    nc.compile()
    return nc


def _get_built():
    global _BUILT
    if _BUILT is None:
        _BUILT = _build()
    return _BUILT


def _host_prep(inputs):
    """Pre-stack/pre-cast the replicated weights and z layout (host-side
    input layout prep; all d-dependent compute stays on device)."""
    bf16 = ml_dtypes.bfloat16
    wmaps = {}
    for li, (fi, fo) in enumerate(DIMS):
        w = np.asarray(inputs[f"W{li}"], np.float32)
        ws = np.zeros((128, fo), dtype=bf16)
        if li == 0:
            for k in range(4):
                ws[32 * k : 32 * k + 3, :] = w.astype(bf16)
        else:
            ws[0:64, :] = w.astype(bf16)
            ws[64:128, :] = w.astype(bf16)
        b = np.asarray(inputs[f"b{li}"], np.float32)
        bs = np.zeros((128, 1), dtype=np.float32)
        if fo == 64:
            bs[0:64, 0] = b
            bs[64:128, 0] = b
        else:
            bs[:, 0] = b
        wmaps[f"Wb{li}"] = ws
        wmaps[f"bb{li}"] = bs
    return wmaps


def make_in_maps(inputs):
    d = np.ascontiguousarray(np.asarray(inputs["distance_matrices_batch"], np.float32))
    z = np.ascontiguousarray(np.asarray(inputs["atomic_numbers_batch"], np.int32))
    B = d.shape[0]
    wmaps = _host_prep(inputs)
    bf16 = ml_dtypes.bfloat16
    in_maps = []
    for c in range(B):
        zf = z[c].astype(np.float32)
        m = {
            "d": d[c],
            "dbf": d[c].astype(bf16),
            "zbc": np.ascontiguousarray(np.broadcast_to(zf[None, :], (128, N))),
            "zf": zf,
        }
        m.update(wmaps)
        in_maps.append(m)
    return in_maps


def kernel(**inputs):
    nc = _get_built()
    in_maps = make_in_maps(inputs)
    B = len(in_maps)
    res = run_bass_kernel_spmd(nc, in_maps, list(range(B)))
    return np.stack([res.results[c]["out"] for c in range(B)], 0)
